# revision 33
# baseline (speedup 1.0000x reference)
"""Trainium2 Bass kernel for CustomYOLOLoss (N=512, S=52, NB=3), 8-core data parallel.

SoA bf16, single X-wide pass (111us vs 235us fp32 baseline).
  - Host: cast inputs to bf16, repack channel-major (15/5 planes of
    [128, 1352] cells per core). Unit-stride planes -> DVE 2x_1p mode,
    half the HBM traffic of fp32.
  - Geometry per box-axis: inter_w = sw - m, enc_w = sw + m with
    m = max(|d|, |dw|); |x| is one tensor_scalar bitwise_and on a uint16
    bitcast (sign-bit mask); all 6 box-axis lanes batched into [P, 6X] ops.
  - bce1_b = ln(1+exp(-c_b)) on the scalar engine;
    bce0 sums recovered as conf + bce1.
  - Responsible box via cross-multiplied IoU comparisons (no per-box
    division); g = iou + u/e = (i*e + u^2)/(u*e) -> a single fp32
    reciprocal_approx_fast after selection.
  - DMAs split across two queues (sync + gpsimd) in
    first-consumer-first order to overlap the DVE ramp; sigmoids run in
    3X pieces as planes land; the scalar engine performs all 6 free-axis
    accumulations (activation Copy accum_out) interleaved with the DVE
    stream so only the G accumulation trails the last vector op.
  - Host combines partial sums (A0, T1, NO, CR, G, NOBJ) into the 4 losses.
  Notes from tuning: gpsimd elementwise is ~17x slower than the DVE and
  its shared SBUF port stalls DVE 2-port ops -- keep it to DMA issue only.
  tensor_tensor_reduce and Pool-engine accumulations fault on HW; use
  vector scalar_tensor_tensor accum_out or scalar Copy accum_out instead.
"""

import numpy as np
import ml_dtypes

import concourse.bass as bass
import concourse.bacc as bacc
import concourse.mybir as mybir
import concourse.tile as tile
from concourse.bass_utils import run_bass_kernel_spmd

F32 = mybir.dt.float32
BF16 = mybir.dt.bfloat16
U16 = mybir.dt.uint16
AF = mybir.ActivationFunctionType
ALU = mybir.AluOpType

N, S, NB = 512, 52, 3
CORES = 8
NPC = N // CORES          # 64 images per core
P = 128
CELLS = NPC * S * S       # 173056
X = CELLS // P            # 1352 cells per partition
NACC = 6                  # A0, T1, NO, CR, G, NOBJ

PERM_IN = [1, 6, 11, 2, 7, 12, 3, 8, 13, 4, 9, 14, 0, 5, 10]
PERM_TG = [1, 2, 3, 4, 0]

_nc_cache = {}


def build_nc():
    if "nc" in _nc_cache:
        return _nc_cache["nc"]
    nc = bacc.Bacc(trn_type="TRN2", target_bir_lowering=False)
    inp = nc.dram_tensor("input", [P, 15 * X], BF16, kind="ExternalInput")
    tgt = nc.dram_tensor("target", [P, 5 * X], BF16, kind="ExternalInput")
    out = nc.dram_tensor("out", [P, NACC], F32, kind="ExternalOutput")

    inp_v = inp[:].rearrange("p (c x) -> p c x", c=15)

    with tile.TileContext(nc) as tc:
        with (
            tc.tile_pool(name="dma", bufs=1) as dma_pool,
            tc.tile_pool(name="work", bufs=1) as work,
            tc.tile_pool(name="accp", bufs=1) as accp,
        ):
            acc = accp.tile([P, NACC], F32)

            # ---- DMAs: two queues (sync: target+conf, tensor: boxes),
            # finest-needed-first for early compute ----
            tg = dma_pool.tile([P, 5 * X], BF16, tag="tgt")
            conf = dma_pool.tile([P, 3 * X], BF16, tag="conf")
            nc.sync.dma_start(tg[:, 2 * X:4 * X], tgt[:, 2 * X:4 * X])
            nc.sync.dma_start(conf[:], inp_v[:, 12:15, :].rearrange(
                "p c x -> p (c x)"))
            nc.sync.dma_start(tg[:, 0:2 * X], tgt[:, 0:2 * X])
            nc.sync.dma_start(tg[:, 4 * X:5 * X], tgt[:, 4 * X:5 * X])
            box = dma_pool.tile([P, 12 * X], BF16, tag="box")
            box_v = box[:].rearrange("p (c x) -> p c x", c=12)
            nc.gpsimd.dma_start(box_v[:, 6:9, :], inp_v[:, 6:9, :])
            nc.gpsimd.dma_start(box_v[:, 9:12, :], inp_v[:, 9:12, :])
            nc.gpsimd.dma_start(box_v[:, 0:3, :], inp_v[:, 0:3, :])
            nc.gpsimd.dma_start(box_v[:, 3:6, :], inp_v[:, 3:6, :])

            TX = tg[:, 0 * X:1 * X]
            TY = tg[:, 1 * X:2 * X]
            TWp = tg[:, 2 * X:3 * X]
            THp = tg[:, 3 * X:4 * X]
            TC = tg[:, 4 * X:5 * X]

            # ---- replicated half-extents + target area (DVE; gpsimd is
            # ~17x slower and its SBUF-port sharing stalls the DVE) ----
            tw2rep = work.tile([P, 6 * X], BF16, tag="tw2rep")
            nc.vector.tensor_scalar(tw2rep[:, 0:X], TWp, 0.5, None, ALU.mult)
            nc.vector.tensor_scalar(tw2rep[:, 3 * X:4 * X], THp, 0.5, None,
                                    ALU.mult)
            nc.vector.tensor_copy(tw2rep[:, X:2 * X], tw2rep[:, 0:X])
            nc.vector.tensor_copy(tw2rep[:, 2 * X:3 * X], tw2rep[:, 0:X])
            nc.vector.tensor_copy(tw2rep[:, 4 * X:5 * X],
                                  tw2rep[:, 3 * X:4 * X])
            nc.vector.tensor_copy(tw2rep[:, 5 * X:6 * X],
                                  tw2rep[:, 3 * X:4 * X])
            areab3 = work.tile([P, 3 * X], BF16, tag="areab3")
            nc.vector.tensor_tensor(areab3[:, 0:X], TWp, THp, ALU.mult)
            nc.vector.tensor_copy(areab3[:, X:2 * X], areab3[:, 0:X])
            nc.vector.tensor_copy(areab3[:, 2 * X:3 * X], areab3[:, 0:X])

            # ---- scalar: sigmoids (in place, 3X pieces, w/h first since
            # their consumer chain is longest), bce1 = ln(1+exp(-c)) ----
            for lo in (6, 9, 0, 3):
                nc.scalar.activation(box[:, lo * X:(lo + 3) * X],
                                     box[:, lo * X:(lo + 3) * X], AF.Sigmoid)
            s0 = dma_pool.tile([P, 3 * X], BF16, tag="s0")
            nc.scalar.activation(s0[:], conf[:], AF.Exp, scale=-1.0)
            nc.scalar.activation(s0[:], s0[:], AF.Ln, bias=1.0)

            Xg = box[:, 0:3 * X]
            Yg = box[:, 3 * X:6 * X]
            Wg = box[:, 6 * X:9 * X]
            Hg = box[:, 9 * X:12 * X]

            q1c = work.tile([P, 2 * X], BF16, tag="q1c")
            q1 = q1c[:, 0:X]
            q1b = q1c[:, X:2 * X]
            c0 = conf[:, 0:X]
            c1 = conf[:, X:2 * X]
            c2 = conf[:, 2 * X:3 * X]
            s0_0 = s0[:, 0:X]
            s0_1 = s0[:, X:2 * X]
            s0_2 = s0[:, 2 * X:3 * X]

            # ---- vector: box geometry (w/h chain first, 3X granularity
            # so compute starts as soon as each sigmoid piece lands) ----
            w2 = work.tile([P, 6 * X], BF16, tag="w2")
            nc.vector.tensor_scalar(w2[:, 0:3 * X], Wg, 0.5, None, ALU.mult)
            nc.vector.tensor_scalar(w2[:, 3 * X:6 * X], Hg, 0.5, None,
                                    ALU.mult)
            dw = work.tile([P, 6 * X], BF16, tag="dw")
            sw = work.tile([P, 6 * X], BF16, tag="sw")
            nc.vector.tensor_tensor(dw[:], w2[:], tw2rep[:], ALU.subtract)
            nc.vector.tensor_tensor(sw[:], w2[:], tw2rep[:], ALU.add)
            # |dw| in place via sign-bit mask
            nc.vector.tensor_scalar(dw[:].bitcast(U16), dw[:].bitcast(U16),
                                    0x7FFF, None, ALU.bitwise_and)
            d_all = work.tile([P, 6 * X], BF16, tag="d_all")
            for b in range(3):
                nc.vector.tensor_tensor(d_all[:, b * X:(b + 1) * X],
                                        Xg[:, b * X:(b + 1) * X], TX,
                                        ALU.subtract)
            for b in range(3):
                nc.vector.tensor_tensor(d_all[:, (3 + b) * X:(4 + b) * X],
                                        Yg[:, b * X:(b + 1) * X], TY,
                                        ALU.subtract)
            # m = max(|d|, |dw|)  (into dw)
            nc.vector.tensor_scalar(d_all[:].bitcast(U16),
                                    d_all[:].bitcast(U16),
                                    0x7FFF, None, ALU.bitwise_and)
            nc.vector.tensor_tensor(dw[:], d_all[:], dw[:], ALU.max)
            # iw (reuse d_all), relu; ew (in place over sw)
            nc.vector.tensor_tensor(d_all[:], sw[:], dw[:], ALU.subtract)
            nc.vector.tensor_scalar(d_all[:], d_all[:], 0.0, None, ALU.max)
            nc.vector.tensor_tensor(sw[:], sw[:], dw[:], ALU.add)
            m_t = work.tile([P, 6 * X], BF16, tag="m_t")

            inter = w2[:, 0:3 * X]
            enc = w2[:, 3 * X:6 * X]
            nc.vector.tensor_tensor(inter, d_all[:, 0:3 * X],
                                    d_all[:, 3 * X:6 * X], ALU.mult)
            nc.vector.tensor_tensor(enc, sw[:, 0:3 * X], sw[:, 3 * X:6 * X],
                                    ALU.mult)
            union = tw2rep[:, 0:3 * X]
            nc.vector.tensor_tensor(union, Wg, Hg, ALU.mult)
            nc.vector.tensor_tensor(union, union, areab3[:], ALU.add)
            nc.vector.tensor_tensor(union, union, inter, ALU.subtract)

            def col(i):
                return acc[:, i:i + 1]

            # ---- q1 = sum_b (c_b + bce1_b); early accumulations ----
            nc.vector.tensor_tensor(q1, c0, c1, ALU.add)
            nc.vector.tensor_tensor(q1, q1, c2, ALU.add)
            nc.vector.tensor_tensor(q1b, s0_0, s0_1, ALU.add)
            nc.vector.tensor_tensor(q1b, q1b, s0_2, ALU.add)
            nc.vector.tensor_tensor(q1, q1, q1b, ALU.add)
            mq = m_t[:, 0:X]
            ms = m_t[:, X:2 * X]
            mc = m_t[:, 2 * X:3 * X]
            mg = m_t[:, 3 * X:4 * X]
            scr = m_t[:, 4 * X:5 * X]
            nc.vector.tensor_tensor(mq, q1, TC, ALU.mult)
            nc.scalar.activation(scr, TC, AF.Copy, accum_out=col(5))
            nc.scalar.activation(scr, q1, AF.Copy, accum_out=col(0))
            nc.scalar.activation(scr, mq, AF.Copy, accum_out=col(1))

            # ---- selection (cross-multiplied iou argmax) ----
            i0, i1, i2 = (inter[:, b * X:(b + 1) * X] for b in range(3))
            u0, u1, u2 = (union[:, b * X:(b + 1) * X] for b in range(3))
            e0, e1, e2 = (enc[:, b * X:(b + 1) * X] for b in range(3))
            pq = dw[:, 0:2 * X]
            p_, q_ = pq[:, 0:X], pq[:, X:2 * X]
            mk = work.tile([P, 2 * X], U16, tag="mk")
            mk1, mk2 = mk[:, 0:X], mk[:, X:2 * X]
            nc.vector.tensor_tensor(p_, i1, u0, ALU.mult)
            nc.vector.tensor_tensor(q_, i0, u1, ALU.mult)
            nc.vector.tensor_tensor(mk1, p_, q_, ALU.is_gt)
            nc.vector.copy_predicated(i0, mk1, i1)
            nc.vector.copy_predicated(u0, mk1, u1)
            nc.vector.tensor_tensor(p_, i2, u0, ALU.mult)
            nc.vector.tensor_tensor(q_, i0, u2, ALU.mult)
            nc.vector.tensor_tensor(mk2, p_, q_, ALU.is_gt)
            nc.vector.copy_predicated(i0, mk2, i2)
            nc.vector.copy_predicated(u0, mk2, u2)

            # conf/bce1 selections + their accumulations (early tail)
            nc.vector.copy_predicated(c0, mk1, c1)
            nc.vector.copy_predicated(c0, mk2, c2)
            nc.vector.copy_predicated(s0_0, mk1, s0_1)
            nc.vector.copy_predicated(s0_0, mk2, s0_2)
            nc.vector.tensor_tensor(mc, c0, TC, ALU.mult)
            nc.vector.tensor_tensor(ms, s0_0, TC, ALU.mult)
            nc.scalar.activation(scr, mc, AF.Copy, accum_out=col(3))
            nc.scalar.activation(scr, ms, AF.Copy, accum_out=col(2))

            # ---- g = iou + union/enc = (i*e + u^2) / (u*e) ----
            nc.vector.copy_predicated(e0, mk1, e1)
            nc.vector.copy_predicated(e0, mk2, e2)
            den = dw[:, 2 * X:3 * X]
            num = dw[:, 3 * X:4 * X]
            nsq = dw[:, 5 * X:6 * X]
            g = dw[:, 4 * X:5 * X]
            nc.vector.tensor_tensor(den, u0, e0, ALU.mult)
            nc.vector.tensor_tensor(num, i0, e0, ALU.mult)
            nc.vector.tensor_tensor(nsq, u0, u0, ALU.mult)
            nc.vector.tensor_tensor(num, num, nsq, ALU.add)
            f32t = work.tile([P, 2 * X], F32, tag="f32t")
            denf, rden = f32t[:, 0:X], f32t[:, X:2 * X]
            nc.vector.tensor_copy(denf, den)
            nc.vector.reciprocal_approx_fast(rden, denf)
            nc.vector.tensor_tensor(g, num, rden, ALU.mult)
            nc.vector.tensor_tensor(mg, g, TC, ALU.mult)
            nc.scalar.activation(scr, mg, AF.Copy, accum_out=col(4))

            nc.gpsimd.dma_start(out[:], acc[:])

    nc.compile()
    _nc_cache["nc"] = nc
    return nc


def make_in_maps(input, target):
    in_maps = []
    for c in range(CORES):
        sl = slice(c * NPC, (c + 1) * NPC)
        a = input[sl].reshape(P, X, 15)[:, :, PERM_IN].transpose(0, 2, 1)
        b = target[sl].reshape(P, X, 5)[:, :, PERM_TG].transpose(0, 2, 1)
        in_maps.append({
            "input": np.ascontiguousarray(a).astype(
                ml_dtypes.bfloat16).reshape(P, 15 * X),
            "target": np.ascontiguousarray(b).astype(
                ml_dtypes.bfloat16).reshape(P, 5 * X),
        })
    return in_maps


def kernel(input, target):
    nc = build_nc()
    in_maps = make_in_maps(input, target)
    res = run_bass_kernel_spmd(nc, in_maps, core_ids=list(range(CORES)))
    total = np.zeros(NACC, dtype=np.float64)
    for r in res.results:
        total += r["out"].reshape(P, NACC).sum(axis=0, dtype=np.float64)
    A0, T1, NO, CR, G, NOBJ = total
    n_obj = NOBJ
    n_noobj = float(N * S * S) - n_obj
    s0r = NO + CR  # sum obj * softplus(c_resp)
    loss_noobj = (A0 - T1) / (n_noobj * NB) + (T1 - s0r) / (n_obj * (NB - 1))
    loss_obj = NO / n_obj
    loss_bbox = (2.0 * n_obj - G) / n_obj
    loss = loss_obj + loss_bbox + loss_noobj
    return (np.float32(loss), np.float32(loss_noobj), np.float32(loss_bbox),
            np.float32(loss_obj))
